# revision 13
# baseline (speedup 1.0000x reference)
"""Causal self-attention Trainium2 kernel (8 NeuronCores, SPMD).

Problem (hardcoded): B=2, T=2048, C=1024, H=16, D=64.
  qkv = x @ W_attn + b_attn ; causal softmax attention ; y @ W_out + b_out

Sharding: core c handles batch b = c//4 and head group g = c%4 (4 heads,
256 channels). Each core computes its heads' attention output and a
partial out-projection [2048, 1024]; the host sums the 4 partials per
batch and adds b_out.

All matmuls run as float32r (full-rate fp32, ~1e-4 rel err).
Layouts are chosen so no on-device transposes are needed:
  - x arrives transposed ([C, T]) from the host.
  - Q^T/K^T are produced directly in [D, T] (heads paired on 128
    partitions) by using W as the stationary operand.
  - scores are computed transposed (S^T[t, q]) so the softmax sum comes
    free from an appended ones-column on V ([V|1] trick), and exp(S^T)
    blocks feed att@V as the moving operand directly.
  - att@V produces O^T[d, q]; normalization multiplies by the
    broadcast reciprocal of the rowsum row (K=1 matmul broadcast).
"""

import sys

if "/opt/trn_rl_repo" not in sys.path:
    sys.path.insert(0, "/opt/trn_rl_repo")

import numpy as np

import concourse.bass as bass
import concourse.mybir as mybir
import concourse.tile as tile
from concourse import bacc, bass_utils

F32 = mybir.dt.float32
F32R = mybir.dt.float32r
MULT = mybir.AluOpType.mult
EXP = mybir.ActivationFunctionType.Exp

B, T, C = 2, 2048, 1024
H, D = 16, 64
HPC = 4          # heads per core
GC = HPC * D     # channels per core's head group (256)
NT = T // 128    # 16 t-tiles
NK = C // 128    # 8 contraction tiles
QCH = 512        # q-chunk width
SCALE = float(1.0 / np.sqrt(D))

_CACHE = {}


def _build(iters=1, phases=3):
    nc = bacc.Bacc("TRN2", target_bir_lowering=False, debug=False,
                   enable_asserts=False, num_devices=8)
    xt_d = nc.dram_tensor("xt", [C, T], F32, kind="ExternalInput").ap()
    wq_d = nc.dram_tensor("wq", [C, GC], F32, kind="ExternalInput").ap()
    wk_d = nc.dram_tensor("wk", [C, GC], F32, kind="ExternalInput").ap()
    wv_d = nc.dram_tensor("wv", [C, GC], F32, kind="ExternalInput").ap()
    bqk_d = nc.dram_tensor("bqk", [128, 4], F32, kind="ExternalInput").ap()
    bv_d = nc.dram_tensor("bv", [128, GC], F32, kind="ExternalInput").ap()
    wo_d = nc.dram_tensor("wo", [GC, C], F32, kind="ExternalInput").ap()
    mask_d = nc.dram_tensor("mask", [128, 128], F32, kind="ExternalInput").ap()
    ones1_d = nc.dram_tensor("ones1", [1, D], F32, kind="ExternalInput").ap()
    onesv_d = nc.dram_tensor("onesv", [128, NT, HPC, 1], F32, kind="ExternalInput").ap()
    y_d = nc.dram_tensor("y", [T, C], F32, kind="ExternalOutput").ap()

    import contextlib

    with tile.TileContext(nc) as tc, nc.allow_low_precision(reason="f32r is 32-bit"):
        loop_ctx = tc.For_i(0, iters, 1) if iters > 1 else contextlib.nullcontext()
        with loop_ctx, tc.tile_pool(name="persist", bufs=1) as sb:
            xt = sb.tile([128, NK, T], F32R)
            wq = sb.tile([128, NK, GC], F32R)
            wk = sb.tile([128, NK, GC], F32R)
            wv = sb.tile([128, NK, GC], F32R)
            bqk = sb.tile([128, 4], F32)
            bv = sb.tile([128, GC], F32)
            wo = sb.tile([128, GC // 128, C], F32R)
            mask = sb.tile([128, 128], F32R)
            ones1 = sb.tile([1, D], F32R)
            qt = [sb.tile([128, T], F32R, name=f"qt{i}") for i in range(2)]
            kt = [sb.tile([128, T], F32R, name=f"kt{i}") for i in range(2)]
            vs = sb.tile([128, NT, HPC, D + 1], F32R)
            ot = [sb.tile([128, T], F32R, name=f"ot{i}") for i in range(2)]

            nc.sync.dma_start(out=wq, in_=wq_d.rearrange("(k p) d -> p k d", p=128).bitcast(F32R))
            nc.sync.dma_start(out=wk, in_=wk_d.rearrange("(k p) d -> p k d", p=128).bitcast(F32R))
            xt_src = xt_d.rearrange("(k p) t -> p k t", p=128).bitcast(F32R)
            for k in range(NK):
                nc.sync.dma_start(out=xt[:, k, :], in_=xt_src[:, k, :])
            nc.sync.dma_start(out=wv, in_=wv_d.rearrange("(k p) d -> p k d", p=128).bitcast(F32R))
            nc.sync.dma_start(out=bqk, in_=bqk_d)
            nc.sync.dma_start(out=bv, in_=bv_d)
            nc.sync.dma_start(out=wo, in_=wo_d.rearrange("(k p) e -> p k e", p=128).bitcast(F32R))
            nc.sync.dma_start(out=mask, in_=mask_d.bitcast(F32R))
            nc.sync.dma_start(out=ones1, in_=ones1_d.bitcast(F32R))
            nc.sync.dma_start(out=vs[:, :, :, D:D + 1], in_=onesv_d.bitcast(F32R))

            # ---- Phase 1 + 2: QKV projections and attention, overlapped ----
            def proj_qt_kt(ps1, half):
                for w_sb, b_col, dst in ((wq, half, qt[half]), (wk, 2 + half, kt[half])):
                    for n in range(T // QCH):
                        acc = ps1.tile([128, QCH], F32, tag="acc")
                        for k in range(NK):
                            nc.tensor.matmul(
                                acc,
                                lhsT=w_sb[:, k, 128 * half:128 * (half + 1)],
                                rhs=xt[:, k, QCH * n:QCH * (n + 1)],
                                start=(k == 0), stop=(k == NK - 1))
                        nc.vector.tensor_scalar_add(
                            out=dst[:, QCH * n:QCH * (n + 1)], in0=acc,
                            scalar1=bqk[:, b_col:b_col + 1])

            # Attention chunk, software-pipelined: the att@V matmuls lag the
            # ST/exp stream by LAG strips so the in-order PE stream never
            # stalls waiting for ACT's exp of the current strip.
            LAG = 2

            def attn_strips(pools, h, m0):
                pt_pool, nrm_pool, ps_st, ps_ot, ps_bc = pools
                half, poff = h // 2, 64 * (h % 2)
                qt_h = qt[half][poff:poff + D, :]
                kt_h = kt[half][poff:poff + D, :]
                q0 = 128 * m0
                psum_ot = ps_ot.tile([D + 1, QCH], F32, tag="ot", name="psum_ot")
                pending = []

                def emit_attv(j, pt):
                    sb_off = (j - m0) * 128 if j > m0 else 0
                    w = QCH - sb_off
                    if j >= m0:
                        nc.tensor.matmul(
                            psum_ot[:, sb_off:sb_off + 128],
                            lhsT=vs[:, j, h, :], rhs=pt[:, 0:128],
                            start=(j == 0), stop=True, skip_group_check=True)
                        if w > 128:
                            nc.tensor.matmul(
                                psum_ot[:, sb_off + 128:QCH],
                                lhsT=vs[:, j, h, :], rhs=pt[:, 128:w],
                                start=(j == 0), stop=False, skip_group_check=True)
                    else:
                        nc.tensor.matmul(
                            psum_ot[:, 0:QCH],
                            lhsT=vs[:, j, h, :], rhs=pt[:, 0:QCH],
                            start=(j == 0), stop=False, skip_group_check=True)

                for j in range(m0 + QCH // 128):
                    sb_off = (j - m0) * 128 if j > m0 else 0
                    w = QCH - sb_off
                    psum_st = ps_st.tile([128, QCH], F32, tag="st", name="psum_st")
                    nc.tensor.matmul(
                        psum_st[:, 0:w],
                        lhsT=kt_h[:, 128 * j:128 * (j + 1)],
                        rhs=qt_h[:, q0 + sb_off:q0 + QCH],
                        start=True, stop=True)
                    pt = pt_pool.tile([128, QCH], F32R, tag="pt", name="pt")
                    nc.scalar.activation(out=pt[:, 0:w], in_=psum_st[:, 0:w],
                                         func=EXP, scale=SCALE)
                    if j >= m0:
                        nc.vector.tensor_tensor(
                            out=pt[:, 0:128], in0=pt[:, 0:128], in1=mask, op=MULT)
                    pending.append((j, pt))
                    if len(pending) > LAG:
                        emit_attv(*pending.pop(0))
                for args in pending:
                    emit_attv(*args)
                return psum_ot, half, poff, q0

            def attn_norm(pools, state):
                pt_pool, nrm_pool, ps_st, ps_ot, ps_bc = pools
                psum_ot, half, poff, q0 = state
                rs_recip = nrm_pool.tile([1, QCH], F32R, tag="rs", name="rs_recip")
                nc.vector.reciprocal(out=rs_recip, in_=psum_ot[D:D + 1, :])
                psum_bc = ps_bc.tile([D, QCH], F32, tag="bc", name="psum_bc")
                nc.tensor.matmul(psum_bc, lhsT=ones1, rhs=rs_recip,
                                 start=True, stop=True)
                bc_sb = nrm_pool.tile([D, QCH], F32, tag="bcs", name="bc_sb")
                nc.vector.tensor_copy(out=bc_sb, in_=psum_bc)
                nc.vector.tensor_tensor(
                    out=ot[half][poff:poff + D, q0:q0 + QCH],
                    in0=psum_ot[0:D, :], in1=bc_sb, op=MULT)

            def outproj_block(ps_mm, ystage, m0):
                for i in range(m0, m0 + QCH // 128):
                    for n in range(C // QCH):
                        acc = ps_mm.tile([128, QCH], F32, tag="acc", name="acc")
                        for half in range(2):
                            nc.tensor.matmul(
                                acc,
                                lhsT=ot[half][:, 128 * i:128 * (i + 1)],
                                rhs=wo[:, half, QCH * n:QCH * (n + 1)],
                                start=(half == 0), stop=(half == 1))
                        yt = ystage.tile([128, QCH], F32, tag="yt", name="yt")
                        nc.vector.tensor_copy(out=yt, in_=acc)
                        nc.sync.dma_start(
                            out=y_d[128 * i:128 * (i + 1), QCH * n:QCH * (n + 1)],
                            in_=yt)

            with tc.tile_pool(name="ps_mm", bufs=2, space="PSUM") as ps_mm, \
                 tc.tile_pool(name="ystage", bufs=3) as ystage:
                proj_qt_kt(ps_mm, 0)
                with tc.tile_pool(name="ps1v", bufs=2, space="PSUM") as ps1v:
                    for j in range(NT):
                        accv = ps1v.tile([128, GC], F32, tag="accv")
                        for k in range(NK):
                            nc.tensor.matmul(
                                accv,
                                lhsT=xt[:, k, 128 * j:128 * (j + 1)],
                                rhs=wv[:, k, :],
                                start=(k == 0), stop=(k == NK - 1))
                        nc.vector.tensor_tensor(
                            out=vs[:, j, :, 0:D],
                            in0=accv.rearrange("p (h d) -> p h d", h=HPC),
                            in1=bv.rearrange("p (h d) -> p h d", h=HPC),
                            op=mybir.AluOpType.add)
                with tc.tile_pool(name="pt_pool", bufs=6) as pt_pool, \
                     tc.tile_pool(name="nrm_pool", bufs=2) as nrm_pool, \
                     tc.tile_pool(name="ps_st", bufs=3, space="PSUM") as ps_st, \
                     tc.tile_pool(name="ps_ot", bufs=2, space="PSUM") as ps_ot, \
                     tc.tile_pool(name="ps_bc", bufs=1, space="PSUM") as ps_bc:
                    pools = (pt_pool, nrm_pool, ps_st, ps_ot, ps_bc)
                    # pair 0 attention (emitted before half-1 proj so it
                    # takes PE priority as soon as deps are ready; half-1
                    # proj fills PE gaps while ACT/DVE work on pair 0).
                    # Each task's normalize is deferred past the next task's
                    # strips to keep the PE stream stall-free.
                    if phases >= 2:
                        tasks = [(h, m0) for m0 in range(0, NT, QCH // 128)
                                 for h in (0, 1)]
                        prev = None
                        for t in tasks:
                            state = attn_strips(pools, *t)
                            if prev is not None:
                                attn_norm(pools, prev)
                            prev = state
                        attn_norm(pools, prev)
                    proj_qt_kt(ps_mm, 1)
                    if phases >= 2:
                        prev = None
                        prev_m0 = None
                        for m0 in range(0, NT, QCH // 128):
                            for h in (2, 3):
                                state = attn_strips(pools, h, m0)
                                if prev is not None:
                                    attn_norm(pools, prev)
                                prev = state
                            if phases >= 3 and prev_m0 is not None:
                                outproj_block(ps_mm, ystage, prev_m0)
                            prev_m0 = m0
                        attn_norm(pools, prev)
                        if phases >= 3:
                            outproj_block(ps_mm, ystage, prev_m0)
    nc.compile()
    return nc


def _get_nc():
    if "nc" not in _CACHE:
        _CACHE["nc"] = _build()
    return _CACHE["nc"]


def make_in_maps(x, W_attn, b_attn, W_out):
    """Per-core input dicts for the SPMD kernel."""
    x = np.asarray(x, dtype=np.float32)
    W_attn = np.asarray(W_attn, dtype=np.float32)
    b_attn = np.asarray(b_attn, dtype=np.float32)
    W_out = np.asarray(W_out, dtype=np.float32)
    mask = np.triu(np.ones((128, 128), np.float32))
    ones1 = np.ones((1, D), np.float32)
    onesv = np.ones((128, NT, HPC, 1), np.float32)
    in_maps = []
    for c in range(8):
        b, g = divmod(c, 4)
        sl = slice(g * GC, (g + 1) * GC)
        bq = b_attn[0 * C:][sl].reshape(2, 128).T          # [128, 2] halves
        bk = b_attn[1 * C:][sl].reshape(2, 128).T
        bqk = np.ascontiguousarray(
            np.stack([bq[:, 0], bq[:, 1], bk[:, 0], bk[:, 1]], axis=1))
        bv = np.tile(b_attn[2 * C:][sl][None, :], (128, 1))
        in_maps.append({
            "xt": np.ascontiguousarray(x[b].T),
            "wq": np.ascontiguousarray(W_attn[:, 0 * C:][:, sl]),
            "wk": np.ascontiguousarray(W_attn[:, 1 * C:][:, sl]),
            "wv": np.ascontiguousarray(W_attn[:, 2 * C:][:, sl]),
            "bqk": bqk,
            "bv": np.ascontiguousarray(bv),
            "wo": np.ascontiguousarray(W_out[sl, :]),
            "mask": mask,
            "ones1": ones1,
            "onesv": onesv,
        })
    return in_maps


def assemble(results, b_out):
    """Sum per-core partials into the full [B, T, C] output."""
    y = np.zeros((B, T, C), np.float32)
    for c in range(8):
        y[c // 4] += results[c]["y"]
    y += np.asarray(b_out, dtype=np.float32)[None, None, :]
    return y


def kernel(x, W_attn, b_attn, W_out, b_out):
    nc = _get_nc()
    in_maps = make_in_maps(x, W_attn, b_attn, W_out)
    res = bass_utils.run_bass_kernel_spmd(nc, in_maps, core_ids=list(range(8)))
    return assemble(res.results, b_out)
